# revision 36
# baseline (speedup 1.0000x reference)
"""Trainium2 Bass kernel for nn_CLFormer (3-block linear-attention transformer).

Sharding: pure data parallel — batch 32 split as 4 per NeuronCore across 8
cores; all parameters replicated; outputs concatenated.

Per-core layout: the 4 local batches x 32 channels are packed onto the 128
SBUF partitions ("channel-major" [128=4bx32c, L]). The kv-gram and the
k-softmax denominator contract over tokens, so a token-major copy
([128=l, (chunk, bc)]) is produced per block via DRAM round-trip DMA
transpose. Attention-out and FC1 are fused into one matmul via
M1 = diag(1/ksum) @ KV @ W1, built block-diagonally so phase 2 runs one
128-partition matmul per stage instead of four serialized 32x32
tile-positioned ones. All weights are packed host-side (pre-replicated,
pre-block-diagonalized, BN folded, bf16 pre-rounded) into two tensors so
startup is 2 DMAs instead of ~60.
"""
import sys
import numpy as np

for _p in ("/opt/trn_rl_repo", "/root/.axon_site/_ro/trn_rl_repo"):
    if _p not in sys.path:
        sys.path.append(_p)

from contextlib import ExitStack

import concourse.bass as bass
import concourse.mybir as mybir
import bass_rust
from concourse import tile
from concourse.masks import make_identity
from concourse.bass_utils import run_bass_kernel_spmd

F32 = mybir.dt.float32
BF16 = mybir.dt.bfloat16
U32 = mybir.dt.uint32
AF = mybir.ActivationFunctionType
MUL = mybir.AluOpType.mult
ADD = mybir.AluOpType.add

P = 128
B_LOC = 4            # batches per core
C = 32               # channels
L = 16384            # sequence length
NB = 3               # transformer blocks
DOUT = 10
HEADS = 4
DH = 8
BN_EPS = 1e-5

SLC = 4096           # slice width (tokens per pipeline slice)
NSL = L // SLC       # 4 slices
NCH = SLC // 128     # 32 chunks per slice
NZ = L // 512        # 32 z-slices
EXT = 129            # chunk pitch in ones-extended token-major tiles
NPR = L // 1024      # 16 z-pairs

# host-packed weight layouts
# bf16 pack (columns):
#   [0,384)    W1bd blocks 0..2 (128 cols each)
#   [384,768)  W2bd blocks 0..2
#   [768,896)  headmask
#   [896,898)  ones, pad
NBF = 898            # bf16 cols (even)
# f32 pack (columns):
#   0..2 b1 | 3..5 b2 | 6 svecL | 7 tvec | 8..39 Whrep | 40..49 Wfrep | 50 bf
NF32 = 51


# ---------------------------------------------------------------- waitfix --
_WF_SKIP = {"InstEventSemaphore"}
_wf_ctr = [0]


def _fix_sync_waits(nc):
    """Hoist excess sync waits onto InstEventSemaphore (this walrus build
    accepts only 1 wait per instruction). The event-sem executes on the same
    engine stream immediately before, preserving semantics."""
    for fn in nc.m.functions:
        new_blocks = []
        for blk in fn.blocks:
            out = []
            for ins in blk.instructions:
                tname = type(ins).__name__
                si = ins.sync_info
                if si is None or tname in _WF_SKIP:
                    out.append(ins)
                    continue
                waits = list(si.on_wait)
                if len(waits) <= 1:
                    out.append(ins)
                    continue
                keep = waits[-1:]
                excess = waits[:-1]
                for i in range(0, len(excess), 2):
                    chunk = excess[i:i + 2]
                    _wf_ctr[0] += 1
                    ev = mybir.InstEventSemaphore(
                        name=f"wfix{_wf_ctr[0]}", ins=[], outs=[])
                    ev.engine = ins.engine
                    ev.sync_info = mybir.SyncInfo(on_wait=chunk, on_update=[])
                    out.append(ev)
                ins.sync_info = mybir.SyncInfo(
                    on_wait=keep, on_update=list(si.on_update))
                out.append(ins)
            nb = bass_rust.BasicBlock(name=blk.name, instructions=out)
            new_blocks.append(nb)
        fn.blocks = new_blocks


# ---------------------------------------------------------------- program --
def build_program():
    nc = bass.Bass()

    x_d = nc.declare_dram_parameter("x", [B_LOC, C, L], F32, isOutput=False)
    wbf_d = nc.declare_dram_parameter("wbf", [P, NBF // 2], U32, isOutput=False)
    wf32_d = nc.declare_dram_parameter("wf32", [P, NF32], F32, isOutput=False)
    out_d = nc.declare_dram_parameter("out", [B_LOC, DOUT], F32, isOutput=True)

    with ExitStack() as ctx:
        tc = ctx.enter_context(tile.TileContext(nc))
        cst = ctx.enter_context(tc.tile_pool(name="cst", bufs=1))
        xst = ctx.enter_context(tc.tile_pool(name="xst", bufs=3))
        hcm = ctx.enter_context(tc.tile_pool(name="hcm", bufs=3))
        hex_ = ctx.enter_context(tc.tile_pool(name="hex", bufs=5))
        etm = ctx.enter_context(tc.tile_pool(name="etm", bufs=4))
        qtm = ctx.enter_context(tc.tile_pool(name="qtm", bufs=2))
        sqp = ctx.enter_context(tc.tile_pool(name="sqp", bufs=2))
        bigq = ctx.enter_context(tc.tile_pool(name="bigq", bufs=2))
        a1p = ctx.enter_context(tc.tile_pool(name="a1p", bufs=3))
        smal = ctx.enter_context(tc.tile_pool(name="smal", bufs=2))
        gps = ctx.enter_context(tc.tile_pool(name="gps", bufs=1, space="PSUM"))
        zps = ctx.enter_context(tc.tile_pool(name="zps", bufs=2, space="PSUM"))
        qps = ctx.enter_context(tc.tile_pool(name="qps", bufs=3, space="PSUM"))

        # ---- weights: two packed DMAs (on ACT's DGE; ACT idle early) ---
        wbfu = cst.tile([P, NBF // 2], U32)
        nc.scalar.dma_start(wbfu[:], wbf_d[:])
        wf32 = cst.tile([P, NF32], F32)
        nc.scalar.dma_start(wf32[:], wf32_d[:])
        wbf = wbfu[:].bitcast(BF16)

        W1bd = [wbf[:, 128 * i:128 * (i + 1)] for i in range(NB)]
        W2bd = [wbf[:, 384 + 128 * i:384 + 128 * (i + 1)] for i in range(NB)]
        headmask = wbf[:, 768:896]
        ones_bf = wbf[:, 896:897]
        b1rep = [wf32[:, i:i + 1] for i in range(NB)]
        b2rep = [wf32[:, 3 + i:4 + i] for i in range(NB)]
        svecL = wf32[:, 6:7]
        tvec = wf32[:, 7:8]
        Whrep = wf32[:, 8:40]
        Wfrep = wf32[:, 40:50]
        bf_s = wf32[:, 50:51]

        ident = cst.tile([P, P], BF16)
        make_identity(nc, ident[:])
        id11 = cst.tile([1, 1], F32)
        nc.vector.memset(id11[:], 1.0)

        pooled_parts = cst.tile([P, NPR], F32)

        x_cm = x_d[:].rearrange("b c l -> (b c) l")

        def new_he_tiles(bi):
            """Allocate a block's token-major tiles ([128, 32*129] bf16,
            ones-extended chunk pitch) and preset the ones columns."""
            tiles = []
            for i in range(NSL):
                he = hex_.tile([P, NCH * EXT], BF16, tag="hex",
                               name=f"he{bi}_{i}")
                hv = he[:].rearrange("p (c l) -> p c l", l=EXT)
                nc.vector.memset(hv[:, :, 128:129], 1.0)
                tiles.append(he)
            return tiles

        def ingest(hn, he_tiles, s, half):
            """PE-transpose a finished channel-major [128,2048] tile into
            token-major chunks of he slice s (2 PSUM trips of 8 chunks)."""
            he = he_tiles[s]
            hv = he[:].rearrange("p (c l) -> p c l", l=EXT)
            for g in range(2):
                qp = qps.tile([P, 1024], BF16, tag="qp")
                for k in range(8):
                    c = 8 * g + k
                    nc.tensor.transpose(
                        qp[:, 128 * k:128 * (k + 1)],
                        hn[:, 128 * c:128 * (c + 1)],
                        ident[:],
                    )
                cbase = 16 * half + 8 * g
                nc.vector.tensor_copy(
                    hv[:, cbase:cbase + 8, 0:128],
                    qp[:].rearrange("p (c l) -> p c l", l=128),
                )

        # block-0 ingest: x -> cast(ACT) -> PE transpose -> he tiles
        h_ext_tiles = new_he_tiles(0)
        for t8 in range(8):
            xs = xst.tile([P, 2048], F32, tag="xs")
            nc.sync.dma_start(xs[:], x_cm[:, 2048 * t8:2048 * (t8 + 1)])
            h0 = hcm.tile([P, 2048], BF16, tag="hcm")
            if t8 % 2 == 0:
                nc.scalar.copy(h0[:], xs[:])
            else:
                nc.vector.tensor_copy(h0[:], xs[:])
            ingest(h0[:], h_ext_tiles, t8 // 2, t8 % 2)

        for blk in range(NB):
            # ============================ phase 1 (token-major) =========
            # G_ext[:, 0:128] = gram E^T h ; G_ext[:, 128] = ksum (ones col)
            G_ps = gps.tile([P, EXT], F32, tag="G")
            q_cm = bigq.tile([P, L], BF16, tag="qcm")
            he_next = new_he_tiles(blk + 1) if blk < NB - 1 else None

            # pass A: exp + gram for every slice, so the M1-build chain
            # (which only needs G) gets scheduler priority over the q-side
            # glue of late slices
            et_tiles = []
            for s in range(NSL):
                he = h_ext_tiles[s]
                hv = he[:].rearrange("p (c l) -> p c l", l=EXT)
                # E = exp(h) (token-major, bf16)
                et = etm.tile([P, SLC], BF16, tag="etm")
                nc.scalar.activation(
                    et[:].rearrange("p (c l) -> p c l", l=128),
                    hv[:, :, 0:128], AF.Exp)
                et_tiles.append(et)
                # gram(+ksum): G_ext += E_chunk.T @ [h_chunk | 1]
                for c in range(NCH):
                    nc.tensor.matmul(
                        G_ps[:],
                        et[:, 128 * c:128 * (c + 1)],
                        he[:, EXT * c:EXT * (c + 1)],
                        start=(s == 0 and c == 0),
                        stop=(s == NSL - 1 and c == NCH - 1),
                    )

            def q_glue(s):
                et = et_tiles[s]
                # q-softmax denominator: segmented sum over d (free dim)
                sq = sqp.tile([P, NCH * 16], F32, tag="sq")
                nc.vector.reduce_sum(
                    sq[:],
                    et[:].rearrange("p (c g d) -> p c g d", g=16, d=DH),
                    axis=mybir.AxisListType.X,
                )
                rq = sqp.tile([P, NCH * 16], F32, tag="rq")
                nc.vector.reciprocal(rq[:], sq[:])
                # q = E * (1/sq) broadcast over d (GPSIMD), 2048-halves to
                # shorten the chain into the q transposes
                qt = qtm.tile([P, SLC], BF16, tag="qtm")
                for hh in range(2):
                    nc.gpsimd.tensor_tensor(
                        qt[:, 2048 * hh:2048 * (hh + 1)]
                            .rearrange("p (c g d) -> p c g d", g=16, d=DH),
                        et[:, 2048 * hh:2048 * (hh + 1)]
                            .rearrange("p (c g d) -> p c g d", g=16, d=DH),
                        rq[:, 256 * hh:256 * (hh + 1)]
                            .rearrange("p (c g) -> p c g", g=16)
                            .unsqueeze(-1).broadcast_to([P, NCH // 2, 16, DH]),
                        op=MUL,
                    )
                # transpose q to channel-major via PE
                for g in range(NCH // 8):
                    qp = qps.tile([P, 1024], BF16, tag="qp")
                    for k in range(8):
                        c = 8 * g + k
                        nc.tensor.transpose(
                            qp[:, 128 * k:128 * (k + 1)],
                            qt[:, 128 * c:128 * (c + 1)],
                            ident[:],
                        )
                    nc.vector.tensor_copy(
                        q_cm[:, SLC * s + 1024 * g: SLC * s + 1024 * (g + 1)],
                        qp[:],
                    )

            # q-glue for all but the last slice (pipeline as exps land)
            for s in range(NSL - 1):
                q_glue(s)

            # ============================ M1 build ======================
            # ksum column comes straight out of the gram (col 128)
            ksC = smal.tile([P, 1], F32, tag="ksC")
            nc.vector.reciprocal(ksC[:], G_ps[:, 128:129])
            # mask to per-head 8x8 diagonal blocks (also batch-diagonal)
            G_sb = smal.tile([P, P], BF16, tag="Gsb")
            nc.vector.tensor_tensor(G_sb[:], G_ps[:, 0:128], headmask, op=MUL)
            # block-diag transpose: DVE 32x32 block transpose (off-diag
            # blocks are zero, so per-32-block transpose == per-strip
            # transpose of the block-diagonal matrix)
            GT_sb = smal.tile([P, P], BF16, tag="gtsb")
            nc.vector.transpose(GT_sb[:], G_sb[:])
            # M1u = G-strips @ W1 as one block-diag matmul
            M1u_t = zps.tile([P, 1024], F32, tag="z")
            nc.tensor.matmul(M1u_t[:, 0:128], GT_sb[:], W1bd[blk])
            M1 = smal.tile([P, P], BF16, tag="m1")
            nc.vector.tensor_scalar_mul(M1[:], M1u_t[:, 0:128], ksC[:])

            # ============================ phase 2 (channel-major) =======
            last = blk == NB - 1
            for tp in range(NPR):
                if tp == 4:
                    q_glue(NSL - 1)
                z1 = zps.tile([P, 1024], F32, tag="z")
                for hh in range(2):
                    nc.tensor.matmul(
                        z1[:, 512 * hh:512 * (hh + 1)], M1[:],
                        q_cm[:, 1024 * tp + 512 * hh:
                             1024 * tp + 512 * (hh + 1)])
                a1 = a1p.tile([P, 1024], BF16, tag="a1")
                nc.scalar.activation(a1[:], z1[:], AF.Gelu, bias=b1rep[blk])
                z2 = zps.tile([P, 1024], F32, tag="z")
                for hh in range(2):
                    nc.tensor.matmul(
                        z2[:, 512 * hh:512 * (hh + 1)], W2bd[blk],
                        a1[:, 512 * hh:512 * (hh + 1)])
                if tp % 2 == 0:
                    hn = hcm.tile([P, 2048], BF16, tag="hcm")
                if last:
                    nc.scalar.activation(
                        hn[:, 1024 * (tp % 2):1024 * (tp % 2 + 1)], z2[:],
                        AF.Gelu, bias=b2rep[blk],
                        accum_out=pooled_parts[:, tp:tp + 1],
                    )
                else:
                    nc.scalar.activation(
                        hn[:, 1024 * (tp % 2):1024 * (tp % 2 + 1)], z2[:],
                        AF.Gelu, bias=b2rep[blk],
                    )
                if tp % 2 == 1 and not last:
                    ingest(hn[:], he_next, tp // 4, (tp // 2) % 2)
            if not last:
                h_ext_tiles = he_next

        # ============================ head ==============================
        psum_ = smal.tile([P, 1], F32, tag="poolsum")
        nc.vector.reduce_sum(psum_[:], pooled_parts[:],
                             axis=mybir.AxisListType.X)
        y_t = zps.tile([P, 1024], F32, tag="z")
        y_ps = y_t[:, 0:1024]
        for b in range(B_LOC):
            sl = slice(C * b, C * (b + 1))
            nc.tensor.matmul(
                y_ps[sl, 0:1], Whrep[sl, :], psum_[sl, :],
                tile_position=(C * b, C * b),
            )
        ybn = smal.tile([P, 1], F32, tag="ybn")
        nc.vector.tensor_scalar(
            ybn[:], y_ps[:, 0:1], svecL, tvec, op0=MUL, op1=ADD,
        )
        yg = smal.tile([P, 1], F32, tag="yg")
        nc.scalar.activation(yg[:], ybn[:], AF.Gelu)
        o_t = zps.tile([P, 1024], F32, tag="z")
        o_ps = o_t[:, 0:1024]
        for b in range(B_LOC):
            nc.tensor.matmul(
                o_ps[C * b:C * b + DOUT, 0:1],
                Wfrep[C * b:C * (b + 1), :],
                yg[C * b:C * (b + 1), :],
                tile_position=(C * b, C * b),
            )
        ob = smal.tile([P, 1], F32, tag="ob")
        for b in range(B_LOC):
            sl = slice(C * b, C * b + DOUT)
            nc.vector.tensor_tensor(ob[sl, :], o_ps[sl, 0:1], bf_s[sl, :],
                                    op=ADD)
        for b in range(B_LOC):
            nc.sync.dma_start(
                out_d[b, :], ob[C * b:C * b + DOUT, 0],
            )

    _fix_sync_waits(nc)
    return nc


# ------------------------------------------------------------- host pack --
def _bf16_bits(a):
    """float32 -> bf16 bit pattern (round to nearest even), as uint16."""
    u = np.ascontiguousarray(a, dtype="<f4").view("<u4")
    r = ((u >> 16) & 1) + np.uint32(0x7FFF)
    return ((u + r) >> 16).astype("<u2")


def _rep4(v):
    """[32]/[32,k] -> [128]/[128,k] replicated across 4 batch strips."""
    return np.concatenate([v] * B_LOC, axis=0)


def pack_weights(a):
    """Build the two packed weight tensors from the raw input dict."""
    bf = np.zeros((P, NBF), np.float32)
    eye4 = np.eye(B_LOC, dtype=np.float32)
    for i in range(NB):
        bf[:, 128 * i:128 * (i + 1)] = np.kron(eye4, a["fcW1"][i])
        bf[:, 384 + 128 * i:384 + 128 * (i + 1)] = np.kron(eye4, a["fcW2"][i])
    idx = np.arange(P)
    bf[:, 768:896] = (idx[:, None] // DH == idx[None, :] // DH)
    bf[:, 896] = 1.0
    bfu = _bf16_bits(bf).view("<u4").reshape(P, NBF // 2)

    f32 = np.zeros((P, NF32), np.float32)
    for i in range(NB):
        f32[:, i] = _rep4(a["fcb1"][i])
        f32[:, 3 + i] = _rep4(a["fcb2"][i])
    svec = a["bn_gamma"] / np.sqrt(a["bn_var"] + BN_EPS)
    f32[:, 6] = _rep4(svec / L)
    f32[:, 7] = _rep4((a["bh"] - a["bn_mean"]) * svec + a["bn_beta"])
    f32[:, 8:40] = _rep4(a["Wh"])
    f32[:, 40:50] = _rep4(a["Wf"])
    bfcol = np.zeros(C, np.float32)
    bfcol[:DOUT] = a["bf"]
    f32[:, 50] = _rep4(bfcol)
    return np.ascontiguousarray(bfu), np.ascontiguousarray(f32)


_NC_CACHE = [None]


def kernel(**inputs) -> np.ndarray:
    arrs = {k: np.asarray(v, dtype=np.float32) for k, v in inputs.items()}
    x = arrs["x"]
    B = x.shape[0]
    n_cores = 8
    bl = B // n_cores

    if _NC_CACHE[0] is None:
        _NC_CACHE[0] = build_program()
    nc = _NC_CACHE[0]

    wbf, wf32 = pack_weights(arrs)
    in_maps = [
        {"x": np.ascontiguousarray(x[bl * i: bl * (i + 1)]),
         "wbf": wbf, "wf32": wf32}
        for i in range(n_cores)
    ]
    res = run_bass_kernel_spmd(nc, in_maps, list(range(n_cores))).results
    return np.concatenate([res[i]["out"] for i in range(n_cores)], axis=0)
